# revision 40
# baseline (speedup 1.0000x reference)
"""Trainium2 Bass kernel for nn_MobileOptimizedSimpleClawMatrix (v5).

All heavy matmuls after the input projections run in fp8e4 with
MatmulPerfMode.DoubleRow (2 k-tiles per instruction at 0.5 cycles/row, 4x
bf16 throughput per output element). Precision is held by hi/lo fp8 splits
(2-term storage, 3-term products), so overall error stays at bf16 level:
l2 ~ 8e-3 on hardware vs the 2e-2 gate.

I/O is fp16 to halve DMA traffic (x, Wv/Wl, out; ~14 MB/core total vs 26):
fp16 is a native 1-cycle/row PE dtype and costs no accuracy here.

Scaling scheme (fp8 subnormals start at 2^-6, so weights are pre-scaled):
  - host ships 16*Wv^T / 16*Wl^T (fp16), fp8 hi/lo pairs of 16*Wo^T,
    biases as 16*bv, 16*bl.
  - phase A: psum = 16*(x @ W^T); vp/lp stored as fp8 hi/lo pairs of
    16*(x@W^T + b) via ACT(bias) for hi and one DVE stt for lo.
  - phase B: sim psum = 256*sim (3-term DoubleRow); softmax via ACT
    Exp(scale=1/256, bias=-max/256) written straight to fp8 (unnormalized
    e8, z accumulated in fp32); vpo stored as fp8 pair of 16*vpo, lpo as
    fp8 pair of (rz/16)*psum = 16*rz*lpo.
  - phase C: out[jb] = (rz/16)[jb]*(e8 @ vpo_pair) + (e8^T @ lpo_pair)/16
    + bo, each stt reading a single PSUM (walrus rejects dual-PSUM reads).
    e8^T comes from PE transposes (fp8 transpose needs element step 2 in
    the PSUM output).

Sharding: batch B=8 across 8 cores, data parallel, params replicated.
"""

import os

os.environ.setdefault("JAX_PLATFORMS", "")

import numpy as np

B = 8
L = 2048  # tokens
D = 768  # feature dim
P = 128
NK = D // P  # 6 chunks over feature dim
NT = L // P  # 16 token blocks

_CACHE = {}


def _build_nc(n_reps: int = 1, dbg: bool = False, mults=(1, 1, 1)):
    from contextlib import ExitStack

    import concourse.bacc as bacc
    import concourse.mybir as mybir
    import concourse.tile as tile
    from concourse.masks import make_identity

    F32 = mybir.dt.float32
    F32R = mybir.dt.float32r
    F16 = mybir.dt.float16
    F8 = mybir.dt.float8e4
    DR = mybir.MatmulPerfMode.DoubleRow
    Exp = mybir.ActivationFunctionType.Exp
    Identity = mybir.ActivationFunctionType.Identity
    Copy = mybir.ActivationFunctionType.Copy
    X = mybir.AxisListType.X
    Mult = mybir.AluOpType.mult
    Add = mybir.AluOpType.add
    Sub = mybir.AluOpType.subtract
    Min = mybir.AluOpType.min

    nc = bacc.Bacc(
        "TRN2", target_bir_lowering=False, debug=False, num_devices=B,
        num_swdge_queues=4,
    )

    # ---- DRAM I/O (per core; host pre-transposes and pre-scales) ----
    xvT = nc.dram_tensor("xvT", [D, L], F16, kind="ExternalInput")
    xlT = nc.dram_tensor("xlT", [D, L], F16, kind="ExternalInput")
    wvT16 = nc.dram_tensor("wvT16", [D, D], F16, kind="ExternalInput")
    wlT16 = nc.dram_tensor("wlT16", [D, D], F16, kind="ExternalInput")
    woh = nc.dram_tensor("woh", [2 * D, D], F8, kind="ExternalInput")
    wol = nc.dram_tensor("wol", [2 * D, D], F8, kind="ExternalInput")
    bv16 = nc.dram_tensor("bv16", [D], F32, kind="ExternalInput")
    bl16 = nc.dram_tensor("bl16", [D], F32, kind="ExternalInput")
    bo = nc.dram_tensor("bo", [D], F32, kind="ExternalInput")
    out = nc.dram_tensor("out", [L, D], F16, kind="ExternalOutput")

    xvT_v = xvT[:].rearrange("(k p) t -> p k t", p=P)
    xlT_v = xlT[:].rearrange("(k p) t -> p k t", p=P)
    wvT_v = wvT16[:].rearrange("(k p) e -> p k e", p=P)
    wlT_v = wlT16[:].rearrange("(k p) e -> p k e", p=P)
    woh_v = woh[:].rearrange("(k p) e -> p k e", p=P)  # [128, 12, 768]
    wol_v = wol[:].rearrange("(k p) e -> p k e", p=P)

    with ExitStack() as ctx:
        tc = ctx.enter_context(tile.TileContext(nc))

        # ---- persistent pools ----
        const = ctx.enter_context(tc.tile_pool(name="const", bufs=1))
        vq_pool = ctx.enter_context(tc.tile_pool(name="vq", bufs=1))
        lq_pool = ctx.enter_context(tc.tile_pool(name="lq", bufs=1))
        wo_pool = ctx.enter_context(tc.tile_pool(name="wo", bufs=1))
        e8_pool = ctx.enter_context(tc.tile_pool(name="e8", bufs=1))
        po_pool = ctx.enter_context(tc.tile_pool(name="po", bufs=1))
        rz_pool = ctx.enter_context(tc.tile_pool(name="rzp", bufs=1))

        ident8 = const.tile([P, P], F8)
        ones1 = const.tile([1, P], F32R)
        with ExitStack() as ictx:
            init = ictx.enter_context(tc.tile_pool(name="init", bufs=1))
            ident_f = init.tile([P, P], F32)
            make_identity(nc, ident_f[:])
            nc.vector.tensor_copy(ident8[:], ident_f[:])
            ones_f = init.tile([1, P], F32)
            nc.gpsimd.memset(ones_f[:], 1.0)
            nc.vector.tensor_copy(ones1[:], ones_f[:])

        for _rep in range(n_reps):
            # fp8 hi/lo pairs of 16*vp^T, 16*lp^T   [P, NK, L]
            vqh = vq_pool.tile([P, NK, L], F8, tag="vqh")
            vql = vq_pool.tile([P, NK, L], F8, tag="vql")
            lqh = lq_pool.tile([P, NK, L], F8, tag="lqh")
            lql = lq_pool.tile([P, NK, L], F8, tag="lql")
            # fp8 pairs of 16*Wo^T (rows 0..5 = wov chunks, 6..11 = wol)
            wo_h = wo_pool.tile([P, 2 * NK, D], F8, tag="wo_h")
            wo_l = wo_pool.tile([P, 2 * NK, D], F8, tag="wo_l")
            # unnormalized exp(sim - max), fp8
            e8 = e8_pool.tile([P, NT, L], F8, tag="e8")
            # fp8 pairs of 16*vpo and 16*rz*lpo
            vpoh = po_pool.tile([P, NT, D], F8, tag="vpoh")
            vpol = po_pool.tile([P, NT, D], F8, tag="vpol")
            lpoh = po_pool.tile([P, NT, D], F8, tag="lpoh")
            lpol = po_pool.tile([P, NT, D], F8, tag="lpol")
            rz_all = rz_pool.tile([P, NT], F32, tag="rz")
            bo_bc = rz_pool.tile([P, D], F32, tag="bo_bc")

            # ============ Phase A: projections -> fp8 pairs ============
            for _ma in range(mults[0]):
              with ExitStack() as actx:
                w_pool = actx.enter_context(tc.tile_pool(name="wA", bufs=1))
                x_pool = actx.enter_context(tc.tile_pool(name="xA", bufs=5))
                pa_pool = actx.enter_context(
                    tc.tile_pool(name="paA", bufs=6, space="PSUM")
                )
                pb_bo = actx.enter_context(
                    tc.tile_pool(name="pbo", bufs=1, space="PSUM")
                )

                wl12 = w_pool.tile([P, NK, D], F16, tag="wl12")
                wv12 = w_pool.tile([P, NK, D], F16, tag="wv12")
                # startup-critical: wl16 split across both queues, one DMA each
                nc.sync.dma_start(wl12[:, 0:3, :], wlT_v[:, 0:3, :])
                nc.gpsimd.dma_start(wl12[:, 3:NK, :], wlT_v[:, 3:NK, :])
                bl_col = w_pool.tile([P, NK], F32, tag="bl_col")
                nc.sync.dma_start(bl_col[:], bl16[:].rearrange("(o p) -> p o", p=P))
                bv_col = w_pool.tile([P, NK], F32, tag="bv_col")
                nc.sync.dma_start(bv_col[:], bv16[:].rearrange("(o p) -> p o", p=P))
                bor = w_pool.tile([1, D], F32R, tag="bor")
                nc.gpsimd.dma_start(bor[:], bo[:].unsqueeze(0))



                # bo broadcast (used in phase C)
                bps = pb_bo.tile([P, D], F32, tag="pbo")
                nc.tensor.matmul(bps[:, 0:512], ones1[:], bor[:, 0:512])
                nc.tensor.matmul(bps[:, 512:D], ones1[:], bor[:, 512:D])
                nc.vector.tensor_copy(bo_bc[:], bps[:])

                def a_load(g, xT_v, q):
                    xg = x_pool.tile([P, NK, 512], F16, tag="xg")
                    q.dma_start(xg[:], xT_v[:, :, g * 512 : (g + 1) * 512])
                    return xg

                def a_compute(g, xg, wt, bcol, dsth, dstl):
                    cols = slice(g * 512, (g + 1) * 512)
                    for me in range(NK):
                        pa = pa_pool.tile([P, 512], F32, tag="pa")
                        for k in range(NK):
                            nc.tensor.matmul(
                                pa[:],
                                wt[:, k, me * P : (me + 1) * P],
                                xg[:, k, :],
                                start=(k == 0), stop=(k == NK - 1),
                            )
                        nc.scalar.activation(
                            dsth[:, me, cols], pa[:], Identity,
                            bias=bcol[:, me : me + 1], scale=1.0,
                        )
                        nc.vector.scalar_tensor_tensor(
                            dstl[:, me, cols], pa[:], bcol[:, me : me + 1],
                            dsth[:, me, cols], op0=Add, op1=Sub,
                        )

                # deep prefetch: issue loads ahead of compute (bufs=5)
                xl_tiles = {g: a_load(g, xlT_v, nc.sync) for g in range(4)}
                xv_tiles = {}
                for g in range(4):
                    if g < 1:
                        xv_tiles[g] = a_load(g, xvT_v, nc.scalar)
                    a_compute(g, xl_tiles.pop(g), wl12, bl_col, lqh, lql)
                    if g == 0:
                        nc.gpsimd.dma_start(wv12[:], wvT_v[:])
                    elif g == 1:
                        nc.gpsimd.dma_start(wo_h[:], woh_v[:])
                for g in range(4):
                    if g + 1 < 4:
                        xv_tiles[g + 1] = a_load(g + 1, xvT_v, nc.scalar)
                    a_compute(g, xv_tiles.pop(g), wv12, bv_col, vqh, vql)
                    if g == 0:
                        nc.gpsimd.dma_start(wo_l[:], wol_v[:])

            # ============ Phase B: sim + softmax -> e8; vpo/lpo ============
            for _mb in range(mults[1]):
              with ExitStack() as bctx:
                stat_pool = bctx.enter_context(tc.tile_pool(name="stat", bufs=16))
                pb_sim = bctx.enter_context(
                    tc.tile_pool(name="pb_sim", bufs=4, space="PSUM")
                )
                pb_proj = bctx.enter_context(
                    tc.tile_pool(name="pb_proj", bufs=2, space="PSUM")
                )

                SIM_PRODS = [(vqh, lqh), (vqh, lql), (vql, lqh)]

                def proj_mm(ib, srch, srcl, wlo_ix):
                    # psum = 256 * proj
                    pp = pb_proj.tile([P, D], F32, tag="pp")
                    for c0, c1 in ((0, 512), (512, D)):
                        n = 0
                        for sh, wh_ix in ((srch, 0), (srch, 1), (srcl, 0)):
                            wt = wo_h if wh_ix == 0 else wo_l
                            for kk in range(3):
                                ks = wlo_ix + 2 * kk
                                nc.tensor.matmul(
                                    pp[:, c0:c1],
                                    sh[:, 2 * kk : 2 * kk + 2,
                                       ib * P : (ib + 1) * P],
                                    wt[:, ks : ks + 2, c0:c1],
                                    start=(n == 0), stop=(n == 8),
                                    perf_mode=DR,
                                )
                                n += 1
                    return pp

                def proj_evac(ib, pp, dsth, dstl, rzs):
                    # rzs is either the float 1/16 or the per-row rz/16 AP;
                    # must be emitted after rzs is written (program order)
                    nc.scalar.activation(dsth[:, ib, :], pp[:], Copy, scale=rzs)
                    nc.vector.scalar_tensor_tensor(
                        dstl[:, ib, :], pp[:], rzs, dsth[:, ib, :],
                        op0=Mult, op1=Sub,
                    )

                for ib in range(NT):
                    # vpo first: covers previous iblk's exp/psum drain
                    pv = proj_mm(ib, vqh, vql, 0)
                    proj_evac(ib, pv, vpoh, vpol, 1.0 / 16.0)
                    # sim row block as 4x [128,512] psums
                    sqs, negs = [], []
                    for q4 in range(4):
                        sq = pb_sim.tile([P, 512], F32, tag="sq")
                        sqs.append(sq)
                        cols = slice(q4 * 512, (q4 + 1) * 512)
                        n = 0
                        for lt, rt in SIM_PRODS:
                            for kk in range(3):
                                nc.tensor.matmul(
                                    sq[:],
                                    lt[:, 2 * kk : 2 * kk + 2,
                                       ib * P : (ib + 1) * P],
                                    rt[:, 2 * kk : 2 * kk + 2, cols],
                                    start=(n == 0), stop=(n == 8),
                                    perf_mode=DR,
                                )
                                n += 1
                        nm = stat_pool.tile([P, 1], F32, tag="negm")
                        negs.append(nm)
                        nc.vector.reduce_max(nm[:], sq[:], axis=X, negate=True)
                    # lpo matmuls (PE work while stats drain); its evac waits
                    # until rz/16 is written below
                    pl = proj_mm(ib, lqh, lql, NK)
                    # negm = -max/256 over all four quarters
                    nm01 = stat_pool.tile([P, 1], F32, tag="nm01")
                    nc.vector.scalar_tensor_tensor(
                        nm01[:], negs[0][:], 1.0, negs[1][:], op0=Mult, op1=Min
                    )
                    nm23 = stat_pool.tile([P, 1], F32, tag="nm23")
                    nc.vector.scalar_tensor_tensor(
                        nm23[:], negs[2][:], 1.0, negs[3][:], op0=Mult, op1=Min
                    )
                    nm256 = stat_pool.tile([P, 1], F32, tag="nm256")
                    nc.vector.scalar_tensor_tensor(
                        nm256[:], nm01[:], 1.0, nm23[:], op0=Mult, op1=Min
                    )
                    negm = stat_pool.tile([P, 1], F32, tag="negm2")
                    nc.vector.tensor_scalar_mul(negm[:], nm256[:], 1.0 / 256.0)
                    zs = []
                    for q4 in range(4):
                        z = stat_pool.tile([P, 1], F32, tag=f"z{q4}")
                        zs.append(z)
                        nc.scalar.activation(
                            e8[:, ib, q4 * 512 : (q4 + 1) * 512], sqs[q4][:],
                            Exp, bias=negm[:], scale=1.0 / 256.0,
                            accum_out=z[:],
                        )
                    z01 = stat_pool.tile([P, 1], F32, tag="z01")
                    nc.vector.scalar_tensor_tensor(
                        z01[:], zs[0][:], 1.0, zs[1][:], op0=Mult, op1=Add
                    )
                    z23 = stat_pool.tile([P, 1], F32, tag="z23")
                    nc.vector.scalar_tensor_tensor(
                        z23[:], zs[2][:], 1.0, zs[3][:], op0=Mult, op1=Add
                    )
                    # rz_all holds rz/16 = 1/(16*z)
                    zsum = stat_pool.tile([P, 1], F32, tag="zsum")
                    nc.vector.scalar_tensor_tensor(
                        zsum[:], z01[:], 1.0, z23[:], op0=Mult, op1=Add
                    )
                    z16 = stat_pool.tile([P, 1], F32, tag="z16")
                    nc.vector.tensor_scalar_mul(z16[:], zsum[:], 16.0)
                    nc.vector.reciprocal(rz_all[:, ib : ib + 1], z16[:])
                    proj_evac(ib, pl, lpoh, lpol, rz_all[:, ib : ib + 1])

            # ============ Phase C: out[jb] ============
            for _mc in range(mults[2]):
              with ExitStack() as cctx:
                attnT_pool = cctx.enter_context(tc.tile_pool(name="attnT", bufs=2))
                outsb_pool = cctx.enter_context(tc.tile_pool(name="outsb", bufs=2))
                pc_tr = cctx.enter_context(
                    tc.tile_pool(name="pc_tr", bufs=2, space="PSUM")
                )
                pc_av = cctx.enter_context(
                    tc.tile_pool(name="pc_av", bufs=2, space="PSUM")
                )
                pc_al = cctx.enter_context(
                    tc.tile_pool(name="pc_al", bufs=1, space="PSUM")
                )

                trans = {}

                def emit_trans(jb):
                    t = attnT_pool.tile([P, NT, P], F8, tag="attnT",
                                        name=f"attnT_{jb}")
                    trans[jb] = t
                    for g8 in range(0, NT, 8):
                        # fp8 transpose needs output element step of 2
                        ptr = pc_tr.tile([P, 8, P, 2], F8, tag="ptr8")
                        for j in range(8):
                            nc.tensor.transpose(
                                ptr[:, j, :, 0],
                                e8[:, jb, (g8 + j) * P : (g8 + j + 1) * P],
                                ident8[:],
                            )
                        nc.scalar.copy(t[:, g8 : g8 + 8, :], ptr[:, :, :, 0])

                emit_trans(0)
                for jb in range(NT):
                    if jb + 1 < NT:
                        emit_trans(jb + 1)
                    t = trans.pop(jb)
                    pav = pc_av.tile([P, D], F32, tag="pav")
                    pal = pc_al.tile([P, D], F32, tag="pal")
                    for c0, c1 in ((0, 512), (512, D)):
                        n = 0
                        for vt in (vpoh, vpol):
                            for jj in range(NT // 2):
                                nc.tensor.matmul(
                                    pav[:, c0:c1],
                                    t[:, 2 * jj : 2 * jj + 2, :],
                                    vt[:, 2 * jj : 2 * jj + 2, c0:c1],
                                    start=(n == 0), stop=(n == NT - 1),
                                    perf_mode=DR,
                                )
                                n += 1
                        n = 0
                        for vt in (lpoh, lpol):
                            for ii in range(NT // 2):
                                nc.tensor.matmul(
                                    pal[:, c0:c1],
                                    e8[:, 2 * ii : 2 * ii + 2,
                                       jb * P : (jb + 1) * P],
                                    vt[:, 2 * ii : 2 * ii + 2, c0:c1],
                                    start=(n == 0), stop=(n == NT - 1),
                                    perf_mode=DR,
                                )
                                n += 1
                    # out = pav*(rz/16) + bo + pal*(1/16); one psum per stt
                    t1 = outsb_pool.tile([P, D], F32, tag="t1")
                    nc.vector.scalar_tensor_tensor(
                        t1[:], pav[:], rz_all[:, jb : jb + 1], bo_bc[:],
                        op0=Mult, op1=Add,
                    )
                    outsb = outsb_pool.tile([P, D], F16, tag="outsb")
                    nc.vector.scalar_tensor_tensor(
                        outsb[:], pal[:], 1.0 / 16.0, t1[:], op0=Mult,
                        op1=Add,
                    )
                    # alternate output blocks across both HWDGE queues
                    oq = nc.sync if jb % 2 == 0 else nc.scalar
                    oq.dma_start(out[jb * P : (jb + 1) * P, :], outsb[:])

    nc.compile()
    return nc


def _build_sharded(nc):
    """Cache a jitted sharded executable so repeat calls skip retracing."""
    import jax
    import concourse.mybir as mybir
    from jax.sharding import Mesh, PartitionSpec
    from jax.experimental.shard_map import shard_map
    from concourse.bass2jax import (
        _bass_exec_p,
        install_neuronx_cc_hook,
        partition_id_tensor,
    )

    install_neuronx_cc_hook()
    partition_name = nc.partition_id_tensor.name if nc.partition_id_tensor else None
    in_names, out_names, out_avals, zero_outs = [], [], [], []
    for alloc in nc.m.functions[0].allocations:
        if not isinstance(alloc, mybir.MemoryLocationSet):
            continue
        name = alloc.memorylocations[0].name
        if alloc.kind == "ExternalInput":
            if name != partition_name:
                in_names.append(name)
        elif alloc.kind == "ExternalOutput":
            shape = tuple(alloc.tensor_shape)
            dtype = mybir.dt.np(alloc.dtype)
            out_names.append(name)
            out_avals.append(jax.core.ShapedArray(shape, dtype))
            zero_outs.append(np.zeros(shape, dtype))
    n_params = len(in_names)
    n_outs = len(out_avals)
    all_in_names = list(in_names) + list(out_names)
    if partition_name is not None:
        all_in_names.append(partition_name)
    donate = tuple(range(n_params, n_params + n_outs))

    def _body(*args):
        operands = list(args)
        if partition_name is not None:
            operands.append(partition_id_tensor())
        return tuple(
            _bass_exec_p.bind(
                *operands,
                out_avals=tuple(out_avals),
                in_names=tuple(all_in_names),
                out_names=tuple(out_names),
                lowering_input_output_aliases=(),
                sim_require_finite=True,
                sim_require_nnan=True,
                nc=nc,
            )
        )

    devices = jax.devices()[:B]
    mesh = Mesh(np.asarray(devices), ("core",))
    sharding = jax.sharding.NamedSharding(mesh, PartitionSpec("core"))
    sharded = jax.jit(
        shard_map(
            _body,
            mesh=mesh,
            in_specs=(PartitionSpec("core"),) * (n_params + n_outs),
            out_specs=(PartitionSpec("core"),) * n_outs,
            check_rep=False,
        ),
        donate_argnums=donate,
        keep_unused=True,
    )

    import jax.numpy as jnp

    zero_shapes = tuple((B * z.shape[0], *z.shape[1:]) for z in zero_outs)
    zero_dtypes = tuple(z.dtype for z in zero_outs)

    @jax.jit
    def _make_zeros():
        return tuple(jnp.zeros(s, d) for s, d in zip(zero_shapes, zero_dtypes))

    def device_zeros():
        return jax.device_put(_make_zeros(), [sharding] * len(zero_shapes))

    return {
        "sharded": sharded,
        "in_names": in_names,
        "out_names": out_names,
        "zero_outs": zero_outs,
        "out_avals": out_avals,
        "sharding": sharding,
        "device_zeros": device_zeros,
    }


def _prep_inputs(vision_features, language_features, Wv, bv, Wl, bl, Wo, bo):
    import ml_dtypes

    E4 = ml_dtypes.float8_e4m3
    wvT16 = np.ascontiguousarray(
        (16.0 * np.asarray(Wv, np.float32).T).astype(np.float16)
    )
    wlT16 = np.ascontiguousarray(
        (16.0 * np.asarray(Wl, np.float32).T).astype(np.float16)
    )
    woT16 = np.ascontiguousarray(16.0 * np.asarray(Wo, np.float32).T)
    woh = woT16.astype(E4)
    wol = (woT16 - woh.astype(np.float32)).astype(E4)
    bv16 = 16.0 * np.asarray(bv, np.float32)
    bl16 = 16.0 * np.asarray(bl, np.float32)
    bo = np.asarray(bo, np.float32)
    vision_features = np.asarray(vision_features, np.float16)
    language_features = np.asarray(language_features, np.float16)

    in_maps = []
    for b in range(B):
        in_maps.append(
            {
                "xvT": np.ascontiguousarray(vision_features[b].T),
                "xlT": np.ascontiguousarray(language_features[b].T),
                "wvT16": wvT16,
                "wlT16": wlT16,
                "woh": woh,
                "wol": wol,
                "bv16": bv16,
                "bl16": bl16,
                "bo": bo,
            }
        )
    return in_maps


def kernel(
    vision_features, language_features, Wv, bv, Wl, bl, Wo, bo
) -> np.ndarray:
    from concourse.bass_utils import run_bass_kernel_spmd

    nc = _CACHE.get("nc")
    if nc is None:
        nc = _build_nc()
        _CACHE["nc"] = nc

    in_maps = _prep_inputs(
        vision_features, language_features, Wv, bv, Wl, bl, Wo, bo
    )

    try:
        ex = _CACHE.get("ex")
        if ex is None:
            ex = _build_sharded(nc)
            _CACHE["ex"] = ex
        concat_in = [
            np.concatenate([m[n] for m in in_maps], axis=0)
            for n in ex["in_names"]
        ]
        out_arrs = ex["sharded"](*concat_in, *ex["device_zeros"]())
        i = ex["out_names"].index("out")
        full = np.asarray(out_arrs[i]).reshape(B, *ex["out_avals"][i].shape)
        return full.astype(np.float32)
    except Exception:
        res = run_bass_kernel_spmd(nc, in_maps, list(range(B)))
        return np.stack([res.results[b]["out"] for b in range(B)]).astype(np.float32)
